# revision 3
# baseline (speedup 1.0000x reference)
"""Trainium2 Bass kernel for AsymmetricQuantLinear — fp8 DoubleRow + rank-1 zero-point.

    x:             [4096, 4096]  f32
    weight_packed: [2048, 11008] int32 (two 4-bit nibbles per value)
    weight_scale:  [11008] f32
    weight_zero:   [11008] f32
    out = x @ ((unpack(weight_packed) - zero) * scale)   -> [4096, 11008] f32

Tensor-parallel over N across 8 NeuronCores (1376 cols each), x replicated.

Math: out = (x̂ @ q)·s − rowsum(x̂) ⊗ (z·s), with x̂ = x_hi + r on corrected
k-tiles. The nibble values q ∈ [0,15] are exact in fp8 e4m3, so the PE streams
RAW q tiles (no on-device dequant at all); an all-ones column appended to the
moving operand makes the PSUM accumulate rowsum(x̂) for free across the same
start/stop group (hi and residual passes included), and the flush applies the
exact rank-1 zero-point term plus the per-column scale in fp32 on the DVE.

The PE runs fp8 perf_mode=DoubleRow (2 k-planes per instruction, 2
MACs/cell/cycle). x is split x = x_hi + r (both e4m3); the first CKP k-pairs
also accumulate r@q, shrinking the x-quantization error from 2.96e-2 (CKP=0)
to 1.95e-2 at (16+CKP)/16 fp8 passes (CKP=9).

Startup/tail tuning: dummy warm-up matmuls on garbage SBUF keep the PE HAM
clock warm through the initial DMA fill; the critical first tiles are DMA'd
in small pieces spread across queues (fair-share per-queue bandwidth makes
one big transfer slow); the wide s/zs broadcast constants are deferred and
split; the ones-column chunk is issued first within each pass so the rank-1
flush term is ready earliest, and the flush + output DMA run per chunk to
shrink the exposed tail after the last matmul.

Host prep is layout/precision only: transpose, nibble unpack, fp8/f32 casts,
pre-tiling so every device DMA is one contiguous run per partition.
"""

import numpy as np
import ml_dtypes

M, K, N = 4096, 4096, 11008
N_CORES = 8
N_SHARD = N // N_CORES          # 1376
P = 128
KT = K // P                     # 32 k-tiles
KP = KT // 2                    # 16 k-pairs (DoubleRow consumes 2 k-tiles)
MSW = 256                       # m columns fetched per x DMA (two 128-wide m-tiles)
MSUP = M // MSW                 # 16
NPAD = N_SHARD + 32             # 1408: pad keeps DoubleRow plane stride 32B-aligned
ONES_COL = N_SHARD              # col 1376 of each q tile holds 1.0 -> rowsum(x̂)
# (n0, mm width, flush width); ones-column chunk first so the rowsum tap
# (its PSUM stop) lands before the other chunks' flushes need it.
MM_CHUNKS = [(1024, 354, 352), (0, 512, 512), (512, 512, 512)]
CPAD = (384, 512, 512)
CKP = 9                         # k-pairs with hi+lo residual correction (0..16)
WARM_MM = 26                    # dummy warm-up matmuls (garbage data, scratch PSUM)

F8 = ml_dtypes.float8_e4m3

_compiled = {}


def _build(ckp):
    import concourse.mybir as mybir
    import concourse.tile as tile
    from concourse import bacc

    f32 = mybir.dt.float32
    f8 = mybir.dt.float8e4
    DR = mybir.MatmulPerfMode.DoubleRow
    ALU = mybir.AluOpType

    nc = bacc.Bacc("TRN2", target_bir_lowering=False, debug=False, num_devices=N_CORES)
    xh = nc.dram_tensor("xh", [MSUP, P, KT, MSW], f8, kind="ExternalInput").ap()
    if ckp:
        xr = nc.dram_tensor("xr", [MSUP, P, 2 * ckp, MSW], f8, kind="ExternalInput").ap()
    q = nc.dram_tensor("q", [KP, P, 2, NPAD], f8, kind="ExternalInput").ap()
    s = nc.dram_tensor("s", [P, N_SHARD], f32, kind="ExternalInput").ap()
    zs = nc.dram_tensor("zs", [P, N_SHARD], f32, kind="ExternalInput").ap()  # -(zero*scale)
    out = nc.dram_tensor("out", [M, N_SHARD], f32, kind="ExternalOutput").ap()

    with tile.TileContext(nc) as tc:
        with (
            tc.tile_pool(name="const", bufs=1) as constp,
            tc.tile_pool(name="wq", bufs=1) as wqp,
            tc.tile_pool(name="xin", bufs=3) as xp,
            tc.tile_pool(name="xrin", bufs=3) as xrp,
            tc.tile_pool(name="ostage", bufs=3) as outp,
            tc.tile_pool(name="psum", space="PSUM", bufs=2) as pp,
        ):
            # --- PE warm-up: matmuls on garbage SBUF into a scratch PSUM ---
            # bank. They have no DMA dependencies, so they issue immediately
            # and keep the HAM activity window busy (full 2.4 GHz clock)
            # while the first real tiles stream in. Results are never read.
            warm_w = constp.tile([P, 2, 128], f8, tag="warmw")
            nc.vector.memset(warm_w[:], 1.0)
            warm_ps = pp.tile([P, 128], f32, tag="warm")
            for _ in range(WARM_MM):
                nc.tensor.matmul(
                    warm_ps[:], warm_w[:], warm_w[:],
                    start=True, stop=True, perf_mode=DR,
                )

            # W tiles are the raw q nibbles (exact in fp8) — DMA only, no
            # dequant. Early tiles are fetched in small pieces: queue
            # bandwidth is fair-shared, so a single large transfer has high
            # latency while small pieces on many queues land fast. x (and
            # residual) transfers are woven into the q stream in k-tile
            # slices so supply tracks the first sweeps' demand. The wide
            # s/zs flush constants are deferred (first needed ~30us in) and
            # split so they never monopolize a queue early.
            w_tiles = [None]

            w0_chunks = []
            for ci, (n0, nw, _) in enumerate(MM_CHUNKS):
                wc = wqp.tile([P, 2, CPAD[ci]], f8, tag=f"w0c{ci}", name="w0c")
                for p0 in range(0, nw, 128):
                    pw = min(128, nw - p0)
                    nc.sync.dma_start(
                        wc[:, :, p0:p0 + pw], q[0, :, :, n0 + p0:n0 + p0 + pw])
                w0_chunks.append(wc)

            def w_slice(kp, ci, n0, nw):
                if kp == 0:
                    return w0_chunks[ci][:, :, 0:nw]
                return w_tiles[kp][:, :, n0:n0 + nw]

            def fetch_q(kp, pieces=1):
                wt = wqp.tile([P, 2, NPAD], f8, tag=f"w{kp}", name="wt")
                pc = NPAD // pieces
                for p0 in range(0, NPAD, pc):
                    nc.sync.dma_start(wt[:, :, p0:p0 + pc], q[kp, :, :, p0:p0 + pc])
                w_tiles.append(wt)

            x0_t = xp.tile([P, KT, MSW], f8, tag="x", name="x_t")
            x1_t = xp.tile([P, KT, MSW], f8, tag="x", name="x_t")
            if ckp:
                xr0_t = xrp.tile([P, 2 * ckp, MSW], f8, tag="xr", name="xr_t")
                xr1_t = xrp.tile([P, 2 * ckp, MSW], f8, tag="xr", name="xr_t")
            # First x slices in 2-ktile x 128-col pieces: the very first
            # matmul needs only x[ktiles 0:2, cols 0:128].
            for xt, msi in ((x0_t, 0), (x1_t, 1)):
                for h in (0, 2):
                    for c0 in (0, 128):
                        nc.sync.dma_start(
                            xt[:, h:h + 2, c0:c0 + 128],
                            xh[msi, :, h:h + 2, c0:c0 + 128])
            fetch_q(1, pieces=4)
            if ckp:
                nc.sync.dma_start(xr0_t[:, 0:4, :], xr[0, :, 0:4, :])
                nc.sync.dma_start(xr1_t[:, 0:4, :], xr[1, :, 0:4, :])
            s_t = constp.tile([P, N_SHARD], f32, tag="s")
            zs_t = constp.tile([P, N_SHARD], f32, tag="zs")
            for kp in range(2, KP):
                fetch_q(kp, pieces=(4 if kp <= 5 else 2))
                if kp % 2 == 0:
                    g = kp // 2
                    if 4 * g < KT:
                        nc.sync.dma_start(
                            x0_t[:, 4 * g:4 * g + 4, :], xh[0, :, 4 * g:4 * g + 4, :])
                        nc.sync.dma_start(
                            x1_t[:, 4 * g:4 * g + 4, :], xh[1, :, 4 * g:4 * g + 4, :])
                elif ckp:
                    g = (kp - 1) // 2
                    if 4 * g < 2 * ckp:
                        ge = min(4 * g + 4, 2 * ckp)
                        nc.sync.dma_start(
                            xr0_t[:, 4 * g:ge, :], xr[0, :, 4 * g:ge, :])
                        nc.sync.dma_start(
                            xr1_t[:, 4 * g:ge, :], xr[1, :, 4 * g:ge, :])
                if kp == 6:
                    for p0 in range(0, N_SHARD, 344):
                        nc.sync.dma_start(s_t[:, p0:p0 + 344], s[:, p0:p0 + 344])
                if kp == 7:
                    for p0 in range(0, N_SHARD, 344):
                        nc.sync.dma_start(zs_t[:, p0:p0 + 344], zs[:, p0:p0 + 344])

            def mm_sweep(ps_chunks, x_t, xr_t, sub):
                lhs = lambda t, kp: t[:, 2 * kp:2 * kp + 2, sub * P:(sub + 1) * P]
                for kp in range(KP):
                    for ci, (n0, nw, _) in enumerate(MM_CHUNKS):
                        nc.tensor.matmul(
                            ps_chunks[ci][:],
                            lhs(x_t, kp),
                            w_slice(kp, ci, n0, nw),
                            start=(kp == 0),
                            stop=(kp == KP - 1 and not ckp),
                            perf_mode=DR,
                        )
                for cp in range(ckp):
                    for ci, (n0, nw, _) in enumerate(MM_CHUNKS):
                        nc.tensor.matmul(
                            ps_chunks[ci][:],
                            lhs(xr_t, cp),
                            w_slice(cp, ci, n0, nw),
                            start=False,
                            stop=(cp == ckp - 1),
                            perf_mode=DR,
                        )

            def mm_sweep_interleaved(psss, x_t, xr_t):
                # Both m-subtiles interleaved in one k-sweep, and each
                # corrected pair's residual MMs issued right after its hi MMs:
                # every q-tile arrival unlocks up to 12 queued MMs.
                for kp in range(KP):
                    for sub in (0, 1):
                        lhsT = x_t[:, 2 * kp:2 * kp + 2, sub * P:(sub + 1) * P]
                        for ci, (n0, nw, _) in enumerate(MM_CHUNKS):
                            nc.tensor.matmul(
                                psss[sub][ci][:],
                                lhsT,
                                w_slice(kp, ci, n0, nw),
                                start=(kp == 0),
                                stop=(kp == KP - 1),
                                perf_mode=DR,
                            )
                    if kp < ckp:
                        for sub in (0, 1):
                            lhsT = xr_t[:, 2 * kp:2 * kp + 2, sub * P:(sub + 1) * P]
                            for ci, (n0, nw, _) in enumerate(MM_CHUNKS):
                                nc.tensor.matmul(
                                    psss[sub][ci][:],
                                    lhsT,
                                    w_slice(kp, ci, n0, nw),
                                    start=False,
                                    stop=False,
                                    perf_mode=DR,
                                )

            def flush(ps_chunks, o_t, msub):
                # PSUM -> SBUF per chunk: per-column scale, then the exact
                # rank-1 zero-point term  o += rowsum(x̂) * (-(zero*scale)),
                # with rowsum(x̂) read from the ones-column of chunk 0 (which
                # stops first), then DMA that chunk out immediately.
                rs = ps_chunks[0][:, 352:353]
                m0 = msub * P
                for ci, (n0, _, fw) in enumerate(MM_CHUNKS):
                    nc.vector.tensor_mul(
                        o_t[:, n0:n0 + fw], ps_chunks[ci][:, 0:fw], s_t[:, n0:n0 + fw])
                    nc.vector.scalar_tensor_tensor(
                        o_t[:, n0:n0 + fw], zs_t[:, n0:n0 + fw], rs,
                        o_t[:, n0:n0 + fw], op0=ALU.mult, op1=ALU.add)
                    nc.sync.dma_start(out[m0:m0 + P, n0:n0 + fw], o_t[:, n0:n0 + fw])

            for msi in range(MSUP):
                if msi == 0:
                    x_t, xr_t = x0_t, (xr0_t if ckp else None)
                elif msi == 1:
                    x_t, xr_t = x1_t, (xr1_t if ckp else None)
                else:
                    x_t = xp.tile([P, KT, MSW], f8, tag="x", name="x_t")
                    for h in (0, KT // 2):
                        nc.sync.dma_start(
                            x_t[:, h:h + KT // 2, :], xh[msi, :, h:h + KT // 2, :])
                    if ckp:
                        xr_t = xrp.tile([P, 2 * ckp, MSW], f8, tag="xr", name="xr_t")
                        nc.sync.dma_start(xr_t[:], xr[msi])
                    else:
                        xr_t = None
                if msi <= 1:
                    o_ts = [outp.tile([P, N_SHARD], f32, tag="o", name="o_t")
                            for _ in (0, 1)]
                    psss = [
                        [pp.tile([P, nw], f32, tag=f"ps{ci}", name=f"ps{ci}")
                         for ci, (n0, nw, _) in enumerate(MM_CHUNKS)]
                        for _ in (0, 1)
                    ]
                    mm_sweep_interleaved(psss, x_t, xr_t)
                    for sub in (0, 1):
                        flush(psss[sub], o_ts[sub], msi * 2 + sub)
                    continue
                for sub in (0, 1):
                    o_t = outp.tile([P, N_SHARD], f32, tag="o")
                    pss = [pp.tile([P, nw], f32, tag=f"ps{ci}", name=f"ps{ci}")
                           for ci, (n0, nw, _) in enumerate(MM_CHUNKS)]
                    mm_sweep(pss, x_t, xr_t, sub)
                    flush(pss, o_t, msi * 2 + sub)

    nc.compile()
    return nc


def _pretile(a, kt_n):
    # [kt_n*P, M] -> [MSUP, P, kt_n, MSW]; element [msi,p,kt,j] = a[kt*P+p, msi*MSW+j]
    return np.ascontiguousarray(a.reshape(kt_n, P, MSUP, MSW).transpose(2, 1, 0, 3))


def _prep_in_maps(x, weight_packed, weight_scale, weight_zero, ckp):
    x = np.asarray(x, dtype=np.float32)
    wp = np.asarray(weight_packed, dtype=np.int32)
    ws = np.asarray(weight_scale, dtype=np.float32)
    wz = np.asarray(weight_zero, dtype=np.float32)

    xt = np.ascontiguousarray(x.T)           # [K, M] f32
    xh8 = xt.astype(F8)                      # [K, M] fp8 hi part
    xh_tiled = _pretile(xh8, KT)
    if ckp:
        kc = 2 * ckp * P
        r8 = (xt[:kc] - xh8[:kc].astype(np.float32)).astype(F8)
        xr_tiled = _pretile(r8, 2 * ckp)

    qfull = np.empty((K, N), dtype=F8)
    qfull[0::2] = (wp & 15).astype(F8)
    qfull[1::2] = ((wp >> 4) & 15).astype(F8)
    zs_neg = (-wz * ws).astype(np.float32)

    in_maps = []
    for c in range(N_CORES):
        n0, n1 = c * N_SHARD, (c + 1) * N_SHARD
        # [KP, P, 2, NPAD]: nibbles, then a ones column (rowsum tap), zero pad.
        qc = np.zeros((KP, P, 2, NPAD), dtype=F8)
        qc[:, :, :, :N_SHARD] = (
            qfull[:, n0:n1].reshape(KP, 2, P, N_SHARD).transpose(0, 2, 1, 3))
        qc[:, :, :, ONES_COL] = F8(1.0)
        m = {
            "xh": xh_tiled,
            "q": qc,
            "s": np.ascontiguousarray(np.broadcast_to(ws[n0:n1][None, :], (P, N_SHARD))),
            "zs": np.ascontiguousarray(
                np.broadcast_to(zs_neg[n0:n1][None, :], (P, N_SHARD))),
        }
        if ckp:
            m["xr"] = xr_tiled
        in_maps.append(m)
    return in_maps


def run(x, weight_packed, weight_scale, weight_zero, trace=False, ckp=CKP, **spmd_kwargs):
    import time

    from concourse.bass_utils import run_bass_kernel_spmd

    if ckp not in _compiled:
        _compiled[ckp] = _build(ckp)
    in_maps = _prep_in_maps(x, weight_packed, weight_scale, weight_zero, ckp)
    last_err = None
    for attempt in range(3):
        try:
            res = run_bass_kernel_spmd(
                _compiled[ckp], in_maps, core_ids=list(range(N_CORES)), trace=trace,
                **spmd_kwargs,
            )
            break
        except Exception as e:  # transient wedged-device faults recover on retry
            last_err = e
            time.sleep(5)
    else:
        raise last_err
    full = np.concatenate([res.results[c]["out"] for c in range(N_CORES)], axis=1)
    return full, res


def kernel(x, weight_packed, weight_scale, weight_zero):
    full, _ = run(x, weight_packed, weight_scale, weight_zero, trace=False)
    return full


# revision 6
# speedup vs baseline: 1.0186x; 1.0186x over previous
"""Trainium2 Bass kernel for AsymmetricQuantLinear — fp8 DoubleRow + rank-1 zero-point.

    x:             [4096, 4096]  f32
    weight_packed: [2048, 11008] int32 (two 4-bit nibbles per value)
    weight_scale:  [11008] f32
    weight_zero:   [11008] f32
    out = x @ ((unpack(weight_packed) - zero) * scale)   -> [4096, 11008] f32

Tensor-parallel over N across 8 NeuronCores (1376 cols each), x replicated.

Math: out = (x̂ @ q)·s − rowsum(x̂) ⊗ (z·s), with x̂ = x_hi + r on corrected
k-tiles. The nibble values q ∈ [0,15] are exact in fp8 e4m3, so the PE streams
RAW q tiles (no on-device dequant at all); an all-ones column appended to the
moving operand makes the PSUM accumulate rowsum(x̂) for free across the same
start/stop group (hi and residual passes included), and the flush applies the
exact rank-1 zero-point term plus the per-column scale in fp32 on the DVE.

The PE runs fp8 perf_mode=DoubleRow (2 k-planes per instruction, 2
MACs/cell/cycle). x is split x = x_hi + r (both e4m3); the first CKP k-pairs
also accumulate r@q, shrinking the x-quantization error from 2.96e-2 (CKP=0)
to 1.95e-2 at (16+CKP)/16 fp8 passes (CKP=9).

Startup/tail tuning: dummy warm-up matmuls on garbage SBUF keep the PE HAM
clock warm through the initial DMA fill; the critical first tiles are DMA'd
in small pieces spread across queues (fair-share per-queue bandwidth makes
one big transfer slow); the wide s/zs broadcast constants are deferred and
split; the ones-column chunk is issued first within each pass so the rank-1
flush term is ready earliest, and the flush + output DMA run per chunk to
shrink the exposed tail after the last matmul.

Host prep is layout/precision only: transpose, nibble unpack, fp8/f32 casts,
pre-tiling so every device DMA is one contiguous run per partition.
"""

import numpy as np
import ml_dtypes

M, K, N = 4096, 4096, 11008
N_CORES = 8
N_SHARD = N // N_CORES          # 1376
P = 128
KT = K // P                     # 32 k-tiles
KP = KT // 2                    # 16 k-pairs (DoubleRow consumes 2 k-tiles)
MSW = 256                       # m columns fetched per x DMA (two 128-wide m-tiles)
MSUP = M // MSW                 # 16
NPAD = N_SHARD + 32             # 1408: pad keeps DoubleRow plane stride 32B-aligned
ONES_COL = N_SHARD              # col 1376 of each q tile holds 1.0 -> rowsum(x̂)
# (n0, mm width, flush width); ones-column chunk first so the rowsum tap
# (its PSUM stop) lands before the other chunks' flushes need it.
MM_CHUNKS = [(1024, 354, 352), (0, 512, 512), (512, 512, 512)]
CPAD = (384, 512, 512)
CKP = 9                         # k-pairs with hi+lo residual correction (0..16)
WARM_MM = 12                    # dummy warm-up matmuls (constant data, scratch PSUM)

F8 = ml_dtypes.float8_e4m3

_compiled = {}


def _build(ckp):
    import concourse.mybir as mybir
    import concourse.tile as tile
    from concourse import bacc

    f32 = mybir.dt.float32
    f8 = mybir.dt.float8e4
    DR = mybir.MatmulPerfMode.DoubleRow
    ALU = mybir.AluOpType

    nc = bacc.Bacc("TRN2", target_bir_lowering=False, debug=False, num_devices=N_CORES)
    xh = nc.dram_tensor("xh", [MSUP, P, KT, MSW], f8, kind="ExternalInput").ap()
    if ckp:
        xr = nc.dram_tensor("xr", [MSUP, P, 2 * ckp, MSW], f8, kind="ExternalInput").ap()
    q = nc.dram_tensor("q", [KP, P, 2, NPAD], f8, kind="ExternalInput").ap()
    s = nc.dram_tensor("s", [P, N_SHARD], f32, kind="ExternalInput").ap()
    zs = nc.dram_tensor("zs", [P, N_SHARD], f32, kind="ExternalInput").ap()  # -(zero*scale)
    out = nc.dram_tensor("out", [M, N_SHARD], f32, kind="ExternalOutput").ap()

    with tile.TileContext(nc) as tc:
        with (
            tc.tile_pool(name="const", bufs=1) as constp,
            tc.tile_pool(name="wq", bufs=1) as wqp,
            tc.tile_pool(name="xin", bufs=3) as xp,
            tc.tile_pool(name="xrin", bufs=3) as xrp,
            tc.tile_pool(name="ostage", bufs=3) as outp,
            tc.tile_pool(name="psum", space="PSUM", bufs=2) as pp,
        ):
            # --- PE warm-up: matmuls on a memset SBUF tile into a scratch ---
            # PSUM bank. Only dependency is a tiny GpSimd memset, so they
            # issue right after engine start and keep the HAM activity
            # window busy (full 2.4 GHz clock) while the first real tiles
            # stream in. Results are never read.
            warm_w = constp.tile([P, 2, 128], f8, tag="warmw")
            nc.gpsimd.memset(warm_w[:], 1.0)
            warm_ps = pp.tile([P, 128], f32, tag="warm")
            for _ in range(WARM_MM):
                nc.tensor.matmul(
                    warm_ps[:], warm_w[:], warm_w[:],
                    start=True, stop=True, perf_mode=DR,
                )

            # W tiles are the raw q nibbles (exact in fp8) — DMA only, no
            # dequant. Big transfers are split along the PARTITION axis
            # (keeps each DMA packet a full contiguous per-partition run —
            # column splits shrink packets and crater DMA throughput) so
            # early tiles land with low latency across parallel queues. x
            # (and residual) transfers are woven into the q stream in
            # k-tile slices so supply tracks the first sweeps' demand. The
            # wide s/zs flush constants are deferred (first needed ~35us
            # in) and split so they never monopolize a queue early.
            w_tiles = [None]

            w0_chunks = []
            for ci, (n0, nw, _) in enumerate(MM_CHUNKS):
                wc = wqp.tile([P, 2, CPAD[ci]], f8, tag=f"w0c{ci}", name="w0c")
                for p0 in (0, 64):
                    nc.sync.dma_start(
                        wc[p0:p0 + 64, :, 0:nw], q[0, p0:p0 + 64, :, n0:n0 + nw])
                w0_chunks.append(wc)

            def w_slice(kp, ci, n0, nw):
                if kp == 0:
                    return w0_chunks[ci][:, :, 0:nw]
                return w_tiles[kp][:, :, n0:n0 + nw]

            def fetch_q(kp, pieces=2):
                wt = wqp.tile([P, 2, NPAD], f8, tag=f"w{kp}", name="wt")
                pc = P // pieces
                for p0 in range(0, P, pc):
                    nc.sync.dma_start(wt[p0:p0 + pc], q[kp, p0:p0 + pc])
                w_tiles.append(wt)

            x0_t = xp.tile([P, KT, MSW], f8, tag="x", name="x_t")
            x1_t = xp.tile([P, KT, MSW], f8, tag="x", name="x_t")
            if ckp:
                xr0_t = xrp.tile([P, 2 * ckp, MSW], f8, tag="xr", name="xr_t")
                xr1_t = xrp.tile([P, 2 * ckp, MSW], f8, tag="xr", name="xr_t")
            # First x slices split by partition halves for low latency.
            for xt, msi in ((x0_t, 0), (x1_t, 1)):
                for p0 in (0, 64):
                    nc.sync.dma_start(
                        xt[p0:p0 + 64, 0:4, :], xh[msi, p0:p0 + 64, 0:4, :])
            fetch_q(1, pieces=4)
            if ckp:
                nc.sync.dma_start(xr0_t[:, 0:4, :], xr[0, :, 0:4, :])
                nc.sync.dma_start(xr1_t[:, 0:4, :], xr[1, :, 0:4, :])
            s_t = constp.tile([P, N_SHARD], f32, tag="s")
            zs_t = constp.tile([P, N_SHARD], f32, tag="zs")
            for kp in range(2, KP):
                fetch_q(kp)
                if kp % 2 == 0:
                    g = kp // 2
                    if 4 * g < KT:
                        nc.sync.dma_start(
                            x0_t[:, 4 * g:4 * g + 4, :], xh[0, :, 4 * g:4 * g + 4, :])
                        nc.sync.dma_start(
                            x1_t[:, 4 * g:4 * g + 4, :], xh[1, :, 4 * g:4 * g + 4, :])
                elif ckp:
                    g = (kp - 1) // 2
                    if 4 * g < 2 * ckp:
                        ge = min(4 * g + 4, 2 * ckp)
                        nc.sync.dma_start(
                            xr0_t[:, 4 * g:ge, :], xr[0, :, 4 * g:ge, :])
                        nc.sync.dma_start(
                            xr1_t[:, 4 * g:ge, :], xr[1, :, 4 * g:ge, :])
                if kp == 6:
                    for p0 in range(0, P, 32):
                        nc.sync.dma_start(s_t[p0:p0 + 32], s[p0:p0 + 32])
                if kp == 7:
                    for p0 in range(0, P, 32):
                        nc.sync.dma_start(zs_t[p0:p0 + 32], zs[p0:p0 + 32])

            def mm_sweep(ps_chunks, x_t, xr_t, sub):
                lhs = lambda t, kp: t[:, 2 * kp:2 * kp + 2, sub * P:(sub + 1) * P]
                for kp in range(KP):
                    for ci, (n0, nw, _) in enumerate(MM_CHUNKS):
                        nc.tensor.matmul(
                            ps_chunks[ci][:],
                            lhs(x_t, kp),
                            w_slice(kp, ci, n0, nw),
                            start=(kp == 0),
                            stop=(kp == KP - 1 and not ckp),
                            perf_mode=DR,
                        )
                for cp in range(ckp):
                    for ci, (n0, nw, _) in enumerate(MM_CHUNKS):
                        nc.tensor.matmul(
                            ps_chunks[ci][:],
                            lhs(xr_t, cp),
                            w_slice(cp, ci, n0, nw),
                            start=False,
                            stop=(cp == ckp - 1),
                            perf_mode=DR,
                        )

            def mm_sweep_interleaved(psss, x_t, xr_t):
                # Both m-subtiles interleaved in one k-sweep, and each
                # corrected pair's residual MMs issued right after its hi MMs:
                # every q-tile arrival unlocks up to 12 queued MMs.
                for kp in range(KP):
                    for sub in (0, 1):
                        lhsT = x_t[:, 2 * kp:2 * kp + 2, sub * P:(sub + 1) * P]
                        for ci, (n0, nw, _) in enumerate(MM_CHUNKS):
                            nc.tensor.matmul(
                                psss[sub][ci][:],
                                lhsT,
                                w_slice(kp, ci, n0, nw),
                                start=(kp == 0),
                                stop=(kp == KP - 1),
                                perf_mode=DR,
                            )
                    if kp < ckp:
                        for sub in (0, 1):
                            lhsT = xr_t[:, 2 * kp:2 * kp + 2, sub * P:(sub + 1) * P]
                            for ci, (n0, nw, _) in enumerate(MM_CHUNKS):
                                nc.tensor.matmul(
                                    psss[sub][ci][:],
                                    lhsT,
                                    w_slice(kp, ci, n0, nw),
                                    start=False,
                                    stop=False,
                                    perf_mode=DR,
                                )

            def flush(ps_chunks, o_t, msub):
                # PSUM -> SBUF per chunk: per-column scale, then the exact
                # rank-1 zero-point term  o += rowsum(x̂) * (-(zero*scale)),
                # with rowsum(x̂) read from the ones-column of chunk 0 (which
                # stops first), then DMA that chunk out immediately.
                rs = ps_chunks[0][:, 352:353]
                m0 = msub * P
                for ci, (n0, _, fw) in enumerate(MM_CHUNKS):
                    nc.vector.tensor_mul(
                        o_t[:, n0:n0 + fw], ps_chunks[ci][:, 0:fw], s_t[:, n0:n0 + fw])
                    nc.vector.scalar_tensor_tensor(
                        o_t[:, n0:n0 + fw], zs_t[:, n0:n0 + fw], rs,
                        o_t[:, n0:n0 + fw], op0=ALU.mult, op1=ALU.add)
                    nc.sync.dma_start(out[m0:m0 + P, n0:n0 + fw], o_t[:, n0:n0 + fw])

            for msi in range(MSUP):
                if msi == 0:
                    x_t, xr_t = x0_t, (xr0_t if ckp else None)
                elif msi == 1:
                    x_t, xr_t = x1_t, (xr1_t if ckp else None)
                else:
                    x_t = xp.tile([P, KT, MSW], f8, tag="x", name="x_t")
                    for p0 in (0, 64):
                        nc.sync.dma_start(x_t[p0:p0 + 64], xh[msi, p0:p0 + 64])
                    if ckp:
                        xr_t = xrp.tile([P, 2 * ckp, MSW], f8, tag="xr", name="xr_t")
                        nc.sync.dma_start(xr_t[:], xr[msi])
                    else:
                        xr_t = None
                if msi <= 1:
                    o_ts = [outp.tile([P, N_SHARD], f32, tag="o", name="o_t")
                            for _ in (0, 1)]
                    psss = [
                        [pp.tile([P, nw], f32, tag=f"ps{ci}", name=f"ps{ci}")
                         for ci, (n0, nw, _) in enumerate(MM_CHUNKS)]
                        for _ in (0, 1)
                    ]
                    mm_sweep_interleaved(psss, x_t, xr_t)
                    for sub in (0, 1):
                        flush(psss[sub], o_ts[sub], msi * 2 + sub)
                    continue
                for sub in (0, 1):
                    o_t = outp.tile([P, N_SHARD], f32, tag="o")
                    pss = [pp.tile([P, nw], f32, tag=f"ps{ci}", name=f"ps{ci}")
                           for ci, (n0, nw, _) in enumerate(MM_CHUNKS)]
                    mm_sweep(pss, x_t, xr_t, sub)
                    flush(pss, o_t, msi * 2 + sub)

    nc.compile()
    return nc


def _pretile(a, kt_n):
    # [kt_n*P, M] -> [MSUP, P, kt_n, MSW]; element [msi,p,kt,j] = a[kt*P+p, msi*MSW+j]
    return np.ascontiguousarray(a.reshape(kt_n, P, MSUP, MSW).transpose(2, 1, 0, 3))


def _prep_in_maps(x, weight_packed, weight_scale, weight_zero, ckp):
    x = np.asarray(x, dtype=np.float32)
    wp = np.asarray(weight_packed, dtype=np.int32)
    ws = np.asarray(weight_scale, dtype=np.float32)
    wz = np.asarray(weight_zero, dtype=np.float32)

    xt = np.ascontiguousarray(x.T)           # [K, M] f32
    xh8 = xt.astype(F8)                      # [K, M] fp8 hi part
    xh_tiled = _pretile(xh8, KT)
    if ckp:
        kc = 2 * ckp * P
        r8 = (xt[:kc] - xh8[:kc].astype(np.float32)).astype(F8)
        xr_tiled = _pretile(r8, 2 * ckp)

    qfull = np.empty((K, N), dtype=F8)
    qfull[0::2] = (wp & 15).astype(F8)
    qfull[1::2] = ((wp >> 4) & 15).astype(F8)
    zs_neg = (-wz * ws).astype(np.float32)

    in_maps = []
    for c in range(N_CORES):
        n0, n1 = c * N_SHARD, (c + 1) * N_SHARD
        # [KP, P, 2, NPAD]: nibbles, then a ones column (rowsum tap), zero pad.
        qc = np.zeros((KP, P, 2, NPAD), dtype=F8)
        qc[:, :, :, :N_SHARD] = (
            qfull[:, n0:n1].reshape(KP, 2, P, N_SHARD).transpose(0, 2, 1, 3))
        qc[:, :, :, ONES_COL] = F8(1.0)
        m = {
            "xh": xh_tiled,
            "q": qc,
            "s": np.ascontiguousarray(np.broadcast_to(ws[n0:n1][None, :], (P, N_SHARD))),
            "zs": np.ascontiguousarray(
                np.broadcast_to(zs_neg[n0:n1][None, :], (P, N_SHARD))),
        }
        if ckp:
            m["xr"] = xr_tiled
        in_maps.append(m)
    return in_maps


def run(x, weight_packed, weight_scale, weight_zero, trace=False, ckp=CKP, **spmd_kwargs):
    import time

    from concourse.bass_utils import run_bass_kernel_spmd

    if ckp not in _compiled:
        _compiled[ckp] = _build(ckp)
    in_maps = _prep_in_maps(x, weight_packed, weight_scale, weight_zero, ckp)
    last_err = None
    for attempt in range(3):
        try:
            res = run_bass_kernel_spmd(
                _compiled[ckp], in_maps, core_ids=list(range(N_CORES)), trace=trace,
                **spmd_kwargs,
            )
            break
        except Exception as e:  # transient wedged-device faults recover on retry
            last_err = e
            time.sleep(5)
    else:
        raise last_err
    full = np.concatenate([res.results[c]["out"] for c in range(N_CORES)], axis=1)
    return full, res


def kernel(x, weight_packed, weight_scale, weight_zero):
    full, _ = run(x, weight_packed, weight_scale, weight_zero, trace=False)
    return full


# revision 8
# speedup vs baseline: 1.0245x; 1.0058x over previous
"""Trainium2 Bass kernel for AsymmetricQuantLinear — fp8 DoubleRow + rank-1 zero-point.

    x:             [4096, 4096]  f32
    weight_packed: [2048, 11008] int32 (two 4-bit nibbles per value)
    weight_scale:  [11008] f32
    weight_zero:   [11008] f32
    out = x @ ((unpack(weight_packed) - zero) * scale)   -> [4096, 11008] f32

Tensor-parallel over N across 8 NeuronCores (1376 cols each), x replicated.

Math: out = (x̂ @ q)·s − rowsum(x̂) ⊗ (z·s), with x̂ = x_hi + r on corrected
k-pairs. The nibble values q ∈ [0,15] are exact in fp8 e4m3, so the PE streams
RAW q tiles (no on-device dequant at all); an all-ones column appended to the
moving operand makes the PSUM accumulate rowsum(x̂) for free across the same
start/stop group (hi and residual passes included), and the flush applies the
exact rank-1 zero-point term plus the per-column scale in fp32 on the DVE.

The PE runs fp8 perf_mode=DoubleRow (2 k-planes per instruction, 2
MACs/cell/cycle). x is split x = x_hi + r (both e4m3); residual passes
also accumulate r@q on a subset of k-pairs.

Residual budget is LOPSIDED on purpose: the first two m-superblocks get full
correction (CKH=16 pairs) and the rest get CKT=8, totalling the same 144
corrected pairs (and the same 1.95e-2 global rel err) as a uniform 9 — but
the head's extra residual matmuls are supply-free PE work that lands exactly
inside the startup window where the shared-HBM DMA fill (~10MB of q/x tiles
at ~245GB/s) would otherwise stall the PE.

Startup/tail details: dummy warm-up matmuls on a memset tile keep the PE HAM
clock warm through the initial fill; big DMAs split along the PARTITION axis
(column splits shrink packets and crater DMA throughput); s/zs flush
constants are DMA'd as single rows and partition-broadcast on GpSimd; the
ones-column chunk is issued first within each pass so the rank-1 flush term
is ready earliest; flush + output DMA run per chunk to shrink the tail.

Host prep is layout/precision only: transpose, nibble unpack, fp8/f32 casts,
pre-tiling so every device DMA is one contiguous run per partition.
"""

import numpy as np
import ml_dtypes

M, K, N = 4096, 4096, 11008
N_CORES = 8
N_SHARD = N // N_CORES          # 1376
P = 128
KT = K // P                     # 32 k-tiles
KP = KT // 2                    # 16 k-pairs (DoubleRow consumes 2 k-tiles)
MSW = 256                       # m columns fetched per x DMA (two 128-wide m-tiles)
MSUP = M // MSW                 # 16
NPAD = N_SHARD + 32             # 1408: pad keeps DoubleRow plane stride 32B-aligned
ONES_COL = N_SHARD              # col 1376 of each q tile holds 1.0 -> rowsum(x̂)
# (n0, mm width, flush width); ones-column chunk first so the rowsum tap
# (its PSUM stop) lands before the other chunks' flushes need it.
MM_CHUNKS = [(1024, 354, 352), (0, 512, 512), (512, 512, 512)]
CPAD = (384, 512, 512)
CKH = 16                        # residual k-pairs on m-superblocks 0-1 (head)
CKT = 8                         # residual k-pairs on m-superblocks 2-15 (tail)
WARM_MM = 12                    # dummy warm-up matmuls (constant data, scratch PSUM)

F8 = ml_dtypes.float8_e4m3

_compiled = {}


def _build(ckh, ckt):
    import concourse.mybir as mybir
    import concourse.tile as tile
    from concourse import bacc

    f32 = mybir.dt.float32
    f8 = mybir.dt.float8e4
    DR = mybir.MatmulPerfMode.DoubleRow
    ALU = mybir.AluOpType

    nc = bacc.Bacc("TRN2", target_bir_lowering=False, debug=False, num_devices=N_CORES)
    xh = nc.dram_tensor("xh", [MSUP, P, KT, MSW], f8, kind="ExternalInput").ap()
    if ckh:
        xrh = nc.dram_tensor("xrh", [2, P, 2 * ckh, MSW], f8, kind="ExternalInput").ap()
    if ckt:
        xrt = nc.dram_tensor(
            "xrt", [MSUP - 2, P, 2 * ckt, MSW], f8, kind="ExternalInput").ap()
    q = nc.dram_tensor("q", [KP, P, 2, NPAD], f8, kind="ExternalInput").ap()
    s = nc.dram_tensor("s", [1, N_SHARD], f32, kind="ExternalInput").ap()
    zs = nc.dram_tensor("zs", [1, N_SHARD], f32, kind="ExternalInput").ap()  # -(zero*scale)
    out = nc.dram_tensor("out", [M, N_SHARD], f32, kind="ExternalOutput").ap()

    with tile.TileContext(nc) as tc:
        with (
            tc.tile_pool(name="const", bufs=1) as constp,
            tc.tile_pool(name="wq", bufs=1) as wqp,
            tc.tile_pool(name="xin", bufs=3) as xp,
            tc.tile_pool(name="xrin", bufs=3) as xrp,
            tc.tile_pool(name="ostage", bufs=3) as outp,
            tc.tile_pool(name="psum", space="PSUM", bufs=2) as pp,
        ):
            # --- PE warm-up: matmuls on a memset SBUF tile into a scratch ---
            # PSUM bank. Only dependency is a tiny GpSimd memset, so they
            # issue right after engine start and keep the HAM activity
            # window busy (full 2.4 GHz clock) while the first real tiles
            # stream in. Results are never read.
            warm_w = constp.tile([P, 2, 128], f8, tag="warmw")
            nc.gpsimd.memset(warm_w[:], 1.0)
            warm_ps = pp.tile([P, 128], f32, tag="warm")
            for _ in range(WARM_MM):
                nc.tensor.matmul(
                    warm_ps[:], warm_w[:], warm_w[:],
                    start=True, stop=True, perf_mode=DR,
                )

            # W tiles are the raw q nibbles (exact in fp8) — DMA only, no
            # dequant. Big transfers are split along the PARTITION axis
            # (keeps each DMA packet a full contiguous per-partition run)
            # so early tiles land with low latency across parallel queues.
            # x (and residual) transfers are woven into the q stream in
            # k-tile slices so supply tracks the head sweeps' demand.
            w_tiles = [None]

            w0_chunks = []
            for ci, (n0, nw, _) in enumerate(MM_CHUNKS):
                wc = wqp.tile([P, 2, CPAD[ci]], f8, tag=f"w0c{ci}", name="w0c")
                for p0 in (0, 64):
                    nc.sync.dma_start(
                        wc[p0:p0 + 64, :, 0:nw], q[0, p0:p0 + 64, :, n0:n0 + nw])
                w0_chunks.append(wc)

            def w_slice(kp, ci, n0, nw):
                if kp == 0:
                    return w0_chunks[ci][:, :, 0:nw]
                return w_tiles[kp][:, :, n0:n0 + nw]

            def fetch_q(kp, pieces=2):
                wt = wqp.tile([P, 2, NPAD], f8, tag=f"w{kp}", name="wt")
                pc = P // pieces
                for p0 in range(0, P, pc):
                    nc.sync.dma_start(wt[p0:p0 + pc], q[kp, p0:p0 + pc])
                w_tiles.append(wt)

            x0_t = xp.tile([P, KT, MSW], f8, tag="x", name="x_t")
            x1_t = xp.tile([P, KT, MSW], f8, tag="x", name="x_t")
            if ckh:
                xr0_t = xrp.tile([P, 2 * ckh, MSW], f8, tag="xrh", name="xr_t")
                xr1_t = xrp.tile([P, 2 * ckh, MSW], f8, tag="xrh", name="xr_t")
            # First x slices split by partition halves for low latency.
            for xt, msi in ((x0_t, 0), (x1_t, 1)):
                for p0 in (0, 64):
                    nc.sync.dma_start(
                        xt[p0:p0 + 64, 0:4, :], xh[msi, p0:p0 + 64, 0:4, :])
            fetch_q(1, pieces=4)
            if ckh:
                for t0 in (0, 4):
                    te = min(t0 + 4, 2 * ckh)
                    if t0 < te:
                        nc.sync.dma_start(xr0_t[:, t0:te, :], xrh[0, :, t0:te, :])
                        nc.sync.dma_start(xr1_t[:, t0:te, :], xrh[1, :, t0:te, :])
            # s/zs flush constants: one 5.5KB row each, broadcast on GpSimd.
            s_row = constp.tile([1, N_SHARD], f32, tag="srow")
            zs_row = constp.tile([1, N_SHARD], f32, tag="zsrow")
            nc.sync.dma_start(s_row[:], s[:])
            nc.sync.dma_start(zs_row[:], zs[:])
            s_t = constp.tile([P, N_SHARD], f32, tag="s")
            zs_t = constp.tile([P, N_SHARD], f32, tag="zs")
            nc.gpsimd.partition_broadcast(s_t[:], s_row[:])
            nc.gpsimd.partition_broadcast(zs_t[:], zs_row[:])
            for kp in range(2, KP):
                fetch_q(kp)
                if kp % 2 == 0:
                    g = kp // 2
                    if 4 * g < KT:
                        nc.sync.dma_start(
                            x0_t[:, 4 * g:4 * g + 4, :], xh[0, :, 4 * g:4 * g + 4, :])
                        nc.sync.dma_start(
                            x1_t[:, 4 * g:4 * g + 4, :], xh[1, :, 4 * g:4 * g + 4, :])
                elif ckh:
                    # residual slice covering pairs kp+1, kp+2 (consumed just
                    # after the matching hi passes in the interleaved sweep)
                    t0 = 2 * kp + 2
                    if t0 < 2 * ckh:
                        te = min(t0 + 4, 2 * ckh)
                        nc.sync.dma_start(
                            xr0_t[:, t0:te, :], xrh[0, :, t0:te, :])
                        nc.sync.dma_start(
                            xr1_t[:, t0:te, :], xrh[1, :, t0:te, :])

            def mm_sweep(ps_chunks, x_t, xr_t, sub):
                lhs = lambda t, kp: t[:, 2 * kp:2 * kp + 2, sub * P:(sub + 1) * P]
                for kp in range(KP):
                    for ci, (n0, nw, _) in enumerate(MM_CHUNKS):
                        nc.tensor.matmul(
                            ps_chunks[ci][:],
                            lhs(x_t, kp),
                            w_slice(kp, ci, n0, nw),
                            start=(kp == 0),
                            stop=(kp == KP - 1 and not ckt),
                            perf_mode=DR,
                        )
                for cp in range(ckt):
                    for ci, (n0, nw, _) in enumerate(MM_CHUNKS):
                        nc.tensor.matmul(
                            ps_chunks[ci][:],
                            lhs(xr_t, cp),
                            w_slice(cp, ci, n0, nw),
                            start=False,
                            stop=(cp == ckt - 1),
                            perf_mode=DR,
                        )

            def mm_sweep_interleaved(psss, x_t, xr_t):
                # Both m-subtiles interleaved in one k-sweep, and each
                # corrected pair's residual MMs issued right after its hi MMs:
                # every q-tile arrival unlocks up to 12 queued MMs.
                for kp in range(KP):
                    for sub in (0, 1):
                        lhsT = x_t[:, 2 * kp:2 * kp + 2, sub * P:(sub + 1) * P]
                        for ci, (n0, nw, _) in enumerate(MM_CHUNKS):
                            nc.tensor.matmul(
                                psss[sub][ci][:],
                                lhsT,
                                w_slice(kp, ci, n0, nw),
                                start=(kp == 0),
                                stop=(kp == KP - 1 and not ckh),
                                perf_mode=DR,
                            )
                    if kp < ckh:
                        for sub in (0, 1):
                            lhsT = xr_t[:, 2 * kp:2 * kp + 2, sub * P:(sub + 1) * P]
                            for ci, (n0, nw, _) in enumerate(MM_CHUNKS):
                                nc.tensor.matmul(
                                    psss[sub][ci][:],
                                    lhsT,
                                    w_slice(kp, ci, n0, nw),
                                    start=False,
                                    stop=(kp == ckh - 1),
                                    perf_mode=DR,
                                )

            def flush(ps_chunks, o_t, msub):
                # PSUM -> SBUF per chunk: per-column scale, then the exact
                # rank-1 zero-point term  o += rowsum(x̂) * (-(zero*scale)),
                # with rowsum(x̂) read from the ones-column of chunk 0 (which
                # stops first), then DMA that chunk out immediately.
                rs = ps_chunks[0][:, 352:353]
                m0 = msub * P
                for ci, (n0, _, fw) in enumerate(MM_CHUNKS):
                    nc.vector.tensor_mul(
                        o_t[:, n0:n0 + fw], ps_chunks[ci][:, 0:fw], s_t[:, n0:n0 + fw])
                    nc.vector.scalar_tensor_tensor(
                        o_t[:, n0:n0 + fw], zs_t[:, n0:n0 + fw], rs,
                        o_t[:, n0:n0 + fw], op0=ALU.mult, op1=ALU.add)
                    nc.sync.dma_start(out[m0:m0 + P, n0:n0 + fw], o_t[:, n0:n0 + fw])

            for msi in range(MSUP):
                if msi == 0:
                    x_t, xr_t = x0_t, (xr0_t if ckh else None)
                elif msi == 1:
                    x_t, xr_t = x1_t, (xr1_t if ckh else None)
                else:
                    x_t = xp.tile([P, KT, MSW], f8, tag="x", name="x_t")
                    for p0 in (0, 64):
                        nc.sync.dma_start(x_t[p0:p0 + 64], xh[msi, p0:p0 + 64])
                    if ckt:
                        xr_t = xrp.tile([P, 2 * ckt, MSW], f8, tag="xrt", name="xr_t")
                        nc.sync.dma_start(xr_t[:], xrt[msi - 2])
                    else:
                        xr_t = None
                if msi <= 1:
                    o_ts = [outp.tile([P, N_SHARD], f32, tag="o", name="o_t")
                            for _ in (0, 1)]
                    psss = [
                        [pp.tile([P, nw], f32, tag=f"ps{ci}", name=f"ps{ci}")
                         for ci, (n0, nw, _) in enumerate(MM_CHUNKS)]
                        for _ in (0, 1)
                    ]
                    mm_sweep_interleaved(psss, x_t, xr_t)
                    for sub in (0, 1):
                        flush(psss[sub], o_ts[sub], msi * 2 + sub)
                    continue
                for sub in (0, 1):
                    o_t = outp.tile([P, N_SHARD], f32, tag="o")
                    pss = [pp.tile([P, nw], f32, tag=f"ps{ci}", name=f"ps{ci}")
                           for ci, (n0, nw, _) in enumerate(MM_CHUNKS)]
                    mm_sweep(pss, x_t, xr_t, sub)
                    flush(pss, o_t, msi * 2 + sub)

    nc.compile()
    return nc


def _pretile(a, kt_n):
    # [kt_n*P, M] -> [MSUP, P, kt_n, MSW]; element [msi,p,kt,j] = a[kt*P+p, msi*MSW+j]
    return np.ascontiguousarray(a.reshape(kt_n, P, MSUP, MSW).transpose(2, 1, 0, 3))


def _prep_in_maps(x, weight_packed, weight_scale, weight_zero, ckh, ckt):
    x = np.asarray(x, dtype=np.float32)
    wp = np.asarray(weight_packed, dtype=np.int32)
    ws = np.asarray(weight_scale, dtype=np.float32)
    wz = np.asarray(weight_zero, dtype=np.float32)

    xt = np.ascontiguousarray(x.T)           # [K, M] f32
    xh8 = xt.astype(F8)                      # [K, M] fp8 hi part
    xh_tiled = _pretile(xh8, KT)
    kmax = 2 * max(ckh, ckt) * P
    if kmax:
        r8 = (xt[:kmax] - xh8[:kmax].astype(np.float32)).astype(F8)
    if ckh:
        xrh_tiled = np.ascontiguousarray(_pretile(r8[:2 * ckh * P], 2 * ckh)[0:2])
    if ckt:
        xrt_tiled = np.ascontiguousarray(_pretile(r8[:2 * ckt * P], 2 * ckt)[2:MSUP])

    qfull = np.empty((K, N), dtype=F8)
    qfull[0::2] = (wp & 15).astype(F8)
    qfull[1::2] = ((wp >> 4) & 15).astype(F8)
    zs_neg = (-wz * ws).astype(np.float32)

    in_maps = []
    for c in range(N_CORES):
        n0, n1 = c * N_SHARD, (c + 1) * N_SHARD
        # [KP, P, 2, NPAD]: nibbles, then a ones column (rowsum tap), zero pad.
        qc = np.zeros((KP, P, 2, NPAD), dtype=F8)
        qc[:, :, :, :N_SHARD] = (
            qfull[:, n0:n1].reshape(KP, 2, P, N_SHARD).transpose(0, 2, 1, 3))
        qc[:, :, :, ONES_COL] = F8(1.0)
        m = {
            "xh": xh_tiled,
            "q": qc,
            "s": np.ascontiguousarray(ws[n0:n1][None, :]),
            "zs": np.ascontiguousarray(zs_neg[n0:n1][None, :]),
        }
        if ckh:
            m["xrh"] = xrh_tiled
        if ckt:
            m["xrt"] = xrt_tiled
        in_maps.append(m)
    return in_maps


def run(x, weight_packed, weight_scale, weight_zero, trace=False,
        ckh=CKH, ckt=CKT, **spmd_kwargs):
    import time

    from concourse.bass_utils import run_bass_kernel_spmd

    key = (ckh, ckt)
    if key not in _compiled:
        _compiled[key] = _build(ckh, ckt)
    in_maps = _prep_in_maps(x, weight_packed, weight_scale, weight_zero, ckh, ckt)
    last_err = None
    for attempt in range(3):
        try:
            res = run_bass_kernel_spmd(
                _compiled[key], in_maps, core_ids=list(range(N_CORES)), trace=trace,
                **spmd_kwargs,
            )
            break
        except Exception as e:  # transient wedged-device faults recover on retry
            last_err = e
            time.sleep(5)
    else:
        raise last_err
    full = np.concatenate([res.results[c]["out"] for c in range(N_CORES)], axis=1)
    return full, res


def kernel(x, weight_packed, weight_scale, weight_zero):
    full, _ = run(x, weight_packed, weight_scale, weight_zero, trace=False)
    return full


# revision 10
# speedup vs baseline: 1.0426x; 1.0177x over previous
"""Trainium2 Bass kernel for AsymmetricQuantLinear — fp8 DoubleRow + rank-1 zero-point.

    x:             [4096, 4096]  f32
    weight_packed: [2048, 11008] int32 (two 4-bit nibbles per value)
    weight_scale:  [11008] f32
    weight_zero:   [11008] f32
    out = x @ ((unpack(weight_packed) - zero) * scale)   -> [4096, 11008] f32

Tensor-parallel over N across 8 NeuronCores (1376 cols each), x replicated.

Math: out = (x̂ @ q)·s − rowsum(x̂) ⊗ (z·s), with x̂ = x_hi + r on corrected
k-pairs. The nibble values q ∈ [0,15] are exact in fp8 e4m3, so the PE streams
RAW q tiles (no on-device dequant at all); an all-ones column appended to the
moving operand makes the PSUM accumulate rowsum(x̂) for free across the same
start/stop group (hi and residual passes included), and the flush applies the
exact rank-1 zero-point term plus the per-column scale in fp32 on the DVE.

The PE runs fp8 perf_mode=DoubleRow (2 k-planes per instruction, 2
MACs/cell/cycle). x is split x = x_hi + r (both e4m3); residual passes
also accumulate r@q on a subset of k-pairs.

Residual budget is LOPSIDED on purpose: the first two m-superblocks get full
correction (CKH=16 pairs) and the rest get CKT=8, totalling the same 144
corrected pairs (and the same 1.95e-2 global rel err) as a uniform 9 — but
the head's extra residual matmuls are supply-free PE work that lands exactly
inside the startup window where the shared-HBM DMA fill (~10MB of q/x tiles
at ~245GB/s) would otherwise stall the PE.

Startup/tail details: dummy warm-up matmuls on a memset tile keep the PE HAM
clock warm through the initial fill; big DMAs split along the PARTITION axis
(column splits shrink packets and crater DMA throughput); s/zs flush
constants are DMA'd as single rows and partition-broadcast on GpSimd; the
ones-column chunk is issued first within each pass so the rank-1 flush term
is ready earliest; flush + output DMA run per chunk to shrink the tail.

Host prep is layout/precision only: transpose, nibble unpack, fp8/f32 casts,
pre-tiling so every device DMA is one contiguous run per partition.
"""

import numpy as np
import ml_dtypes

M, K, N = 4096, 4096, 11008
N_CORES = 8
N_SHARD = N // N_CORES          # 1376
P = 128
KT = K // P                     # 32 k-tiles
KP = KT // 2                    # 16 k-pairs (DoubleRow consumes 2 k-tiles)
MSW = 256                       # m columns fetched per x DMA (two 128-wide m-tiles)
MSUP = M // MSW                 # 16
NPAD = N_SHARD + 32             # 1408: pad keeps DoubleRow plane stride 32B-aligned
ONES_COL = N_SHARD              # col 1376 of each q tile holds 1.0 -> rowsum(x̂)
# (n0, mm width, flush width); ones-column chunk first so the rowsum tap
# (its PSUM stop) lands before the other chunks' flushes need it.
MM_CHUNKS = [(1024, 354, 352), (0, 512, 512), (512, 512, 512)]
CPAD = (384, 512, 512)
CKH = 16                        # residual k-pairs on m-superblocks 0-1 (head)
CKT = 8                         # residual k-pairs on m-superblocks 2-15 (tail)
WARM_MM = 12                    # dummy warm-up matmuls (constant data, scratch PSUM)

F8 = ml_dtypes.float8_e4m3

_compiled = {}


def _build(ckh, ckt):
    import concourse.mybir as mybir
    import concourse.tile as tile
    from concourse import bacc

    f32 = mybir.dt.float32
    f8 = mybir.dt.float8e4
    DR = mybir.MatmulPerfMode.DoubleRow
    ALU = mybir.AluOpType

    nc = bacc.Bacc("TRN2", target_bir_lowering=False, debug=False, num_devices=N_CORES)
    xh = nc.dram_tensor("xh", [MSUP, P, KT, MSW], f8, kind="ExternalInput").ap()
    if ckh:
        xrh = nc.dram_tensor("xrh", [2, P, 2 * ckh, MSW], f8, kind="ExternalInput").ap()
    if ckt:
        xrt = nc.dram_tensor(
            "xrt", [MSUP - 2, P, 2 * ckt, MSW], f8, kind="ExternalInput").ap()
    q = nc.dram_tensor("q", [KP, P, 2, NPAD], f8, kind="ExternalInput").ap()
    s = nc.dram_tensor("s", [1, N_SHARD], f32, kind="ExternalInput").ap()
    zs = nc.dram_tensor("zs", [1, N_SHARD], f32, kind="ExternalInput").ap()  # -(zero*scale)
    out = nc.dram_tensor("out", [M, N_SHARD], f32, kind="ExternalOutput").ap()

    with tile.TileContext(nc) as tc:
        with (
            tc.tile_pool(name="const", bufs=1) as constp,
            tc.tile_pool(name="wq", bufs=1) as wqp,
            tc.tile_pool(name="xin", bufs=3) as xp,
            tc.tile_pool(name="xrin", bufs=3) as xrp,
            tc.tile_pool(name="ostage", bufs=3) as outp,
            tc.tile_pool(name="psum", space="PSUM", bufs=2) as pp,
        ):
            # --- PE warm-up: matmuls on a memset SBUF tile into a scratch ---
            # PSUM bank. Only dependency is a tiny GpSimd memset, so they
            # issue right after engine start and keep the HAM activity
            # window busy (full 2.4 GHz clock) while the first real tiles
            # stream in. Results are never read.
            warm_w = constp.tile([P, 2, 128], f8, tag="warmw")
            nc.gpsimd.memset(warm_w[:], 1.0)
            warm_ps = pp.tile([P, 128], f32, tag="warm")
            for _ in range(WARM_MM):
                nc.tensor.matmul(
                    warm_ps[:], warm_w[:], warm_w[:],
                    start=True, stop=True, perf_mode=DR,
                )

            # W tiles are the raw q nibbles (exact in fp8) — DMA only, no
            # dequant. A single dma_start is already sprayed across all 16
            # SDMA engines, and efficiency grows with transfer size (~65%
            # at 360KB, ~78% at 1MB), so tiles are fetched in the LARGEST
            # units whose arrival still leads consumption: per-chunk for
            # kp0 (gates the first matmul), single tile for kp1, then 2-kp
            # super-tile DMAs, with x / residual batches woven between them
            # so supply tracks the head sweeps' demand in consumption order.
            w_tiles = [None] * KP

            w0_chunks = []
            for ci, (n0, nw, _) in enumerate(MM_CHUNKS):
                wc = wqp.tile([P, 2, CPAD[ci]], f8, tag=f"w0c{ci}", name="w0c")
                nc.sync.dma_start(wc[:, :, 0:nw], q[0, :, :, n0:n0 + nw])
                w0_chunks.append(wc)

            def w_slice(kp, ci, n0, nw):
                if kp == 0:
                    return w0_chunks[ci][:, :, 0:nw]
                return w_tiles[kp][:, :, n0:n0 + nw]

            def fetch_q1():
                wt = wqp.tile([P, 2, NPAD], f8, tag="w1", name="wt")
                nc.sync.dma_start(wt[:], q[1])
                w_tiles[1] = wt

            def fetch_q2(kp0):
                wt = wqp.tile([P, 2, 2, NPAD], f8, tag=f"w{kp0}", name="wt")
                nc.sync.dma_start(
                    wt[:], q[kp0:kp0 + 2].rearrange("a p c d -> p a c d"))
                w_tiles[kp0] = wt[:, 0]
                w_tiles[kp0 + 1] = wt[:, 1]

            x0_t = xp.tile([P, KT, MSW], f8, tag="x", name="x_t")
            x1_t = xp.tile([P, KT, MSW], f8, tag="x", name="x_t")
            if ckh:
                xr0_t = xrp.tile([P, 2 * ckh, MSW], f8, tag="xrh", name="xr_t")
                xr1_t = xrp.tile([P, 2 * ckh, MSW], f8, tag="xrh", name="xr_t")

            def x_batch(t0, te):
                nc.sync.dma_start(x0_t[:, t0:te, :], xh[0, :, t0:te, :])
                nc.sync.dma_start(x1_t[:, t0:te, :], xh[1, :, t0:te, :])

            def xr_batch(t0, te):
                if ckh and t0 < 2 * ckh:
                    te = min(te, 2 * ckh)
                    nc.sync.dma_start(xr0_t[:, t0:te, :], xrh[0, :, t0:te, :])
                    nc.sync.dma_start(xr1_t[:, t0:te, :], xrh[1, :, t0:te, :])

            x_batch(0, 4)
            fetch_q1()
            xr_batch(0, 4)
            fetch_q2(2)
            x_batch(4, 12)
            fetch_q2(4)
            xr_batch(4, 12)
            fetch_q2(6)
            x_batch(12, 20)
            fetch_q2(8)
            xr_batch(12, 20)
            fetch_q2(10)
            x_batch(20, 28)
            fetch_q2(12)
            xr_batch(20, 28)
            fetch_q2(14)
            x_batch(28, KT)
            xr_batch(28, KT)
            # s/zs flush constants: one 5.5KB row each, broadcast on GpSimd.
            s_row = constp.tile([1, N_SHARD], f32, tag="srow")
            zs_row = constp.tile([1, N_SHARD], f32, tag="zsrow")
            nc.sync.dma_start(s_row[:], s[:])
            nc.sync.dma_start(zs_row[:], zs[:])
            s_t = constp.tile([P, N_SHARD], f32, tag="s")
            zs_t = constp.tile([P, N_SHARD], f32, tag="zs")
            nc.gpsimd.partition_broadcast(s_t[:], s_row[:])
            nc.gpsimd.partition_broadcast(zs_t[:], zs_row[:])

            def mm_sweep(ps_chunks, x_t, xr_t, sub):
                lhs = lambda t, kp: t[:, 2 * kp:2 * kp + 2, sub * P:(sub + 1) * P]
                for kp in range(KP):
                    for ci, (n0, nw, _) in enumerate(MM_CHUNKS):
                        nc.tensor.matmul(
                            ps_chunks[ci][:],
                            lhs(x_t, kp),
                            w_slice(kp, ci, n0, nw),
                            start=(kp == 0),
                            stop=(kp == KP - 1 and not ckt),
                            perf_mode=DR,
                        )
                for cp in range(ckt):
                    for ci, (n0, nw, _) in enumerate(MM_CHUNKS):
                        nc.tensor.matmul(
                            ps_chunks[ci][:],
                            lhs(xr_t, cp),
                            w_slice(cp, ci, n0, nw),
                            start=False,
                            stop=(cp == ckt - 1),
                            perf_mode=DR,
                        )

            def mm_sweep_interleaved(psss, x_t, xr_t):
                # Both m-subtiles interleaved in one k-sweep, and each
                # corrected pair's residual MMs issued right after its hi MMs:
                # every q-tile arrival unlocks up to 12 queued MMs.
                for kp in range(KP):
                    for sub in (0, 1):
                        lhsT = x_t[:, 2 * kp:2 * kp + 2, sub * P:(sub + 1) * P]
                        for ci, (n0, nw, _) in enumerate(MM_CHUNKS):
                            nc.tensor.matmul(
                                psss[sub][ci][:],
                                lhsT,
                                w_slice(kp, ci, n0, nw),
                                start=(kp == 0),
                                stop=(kp == KP - 1 and not ckh),
                                perf_mode=DR,
                            )
                    if kp < ckh:
                        for sub in (0, 1):
                            lhsT = xr_t[:, 2 * kp:2 * kp + 2, sub * P:(sub + 1) * P]
                            for ci, (n0, nw, _) in enumerate(MM_CHUNKS):
                                nc.tensor.matmul(
                                    psss[sub][ci][:],
                                    lhsT,
                                    w_slice(kp, ci, n0, nw),
                                    start=False,
                                    stop=(kp == ckh - 1),
                                    perf_mode=DR,
                                )

            def flush(ps_chunks, o_t, msub):
                # PSUM -> SBUF per chunk: per-column scale, then the exact
                # rank-1 zero-point term  o += rowsum(x̂) * (-(zero*scale)),
                # with rowsum(x̂) read from the ones-column of chunk 0 (which
                # stops first), then DMA that chunk out immediately.
                rs = ps_chunks[0][:, 352:353]
                m0 = msub * P
                for ci, (n0, _, fw) in enumerate(MM_CHUNKS):
                    nc.vector.tensor_mul(
                        o_t[:, n0:n0 + fw], ps_chunks[ci][:, 0:fw], s_t[:, n0:n0 + fw])
                    nc.vector.scalar_tensor_tensor(
                        o_t[:, n0:n0 + fw], zs_t[:, n0:n0 + fw], rs,
                        o_t[:, n0:n0 + fw], op0=ALU.mult, op1=ALU.add)
                    nc.sync.dma_start(out[m0:m0 + P, n0:n0 + fw], o_t[:, n0:n0 + fw])

            for msi in range(MSUP):
                if msi == 0:
                    x_t, xr_t = x0_t, (xr0_t if ckh else None)
                elif msi == 1:
                    x_t, xr_t = x1_t, (xr1_t if ckh else None)
                else:
                    x_t = xp.tile([P, KT, MSW], f8, tag="x", name="x_t")
                    nc.sync.dma_start(x_t[:], xh[msi])
                    if ckt:
                        xr_t = xrp.tile([P, 2 * ckt, MSW], f8, tag="xrt", name="xr_t")
                        nc.sync.dma_start(xr_t[:], xrt[msi - 2])
                    else:
                        xr_t = None
                if msi <= 1:
                    o_ts = [outp.tile([P, N_SHARD], f32, tag="o", name="o_t")
                            for _ in (0, 1)]
                    psss = [
                        [pp.tile([P, nw], f32, tag=f"ps{ci}", name=f"ps{ci}")
                         for ci, (n0, nw, _) in enumerate(MM_CHUNKS)]
                        for _ in (0, 1)
                    ]
                    mm_sweep_interleaved(psss, x_t, xr_t)
                    for sub in (0, 1):
                        flush(psss[sub], o_ts[sub], msi * 2 + sub)
                    continue
                for sub in (0, 1):
                    o_t = outp.tile([P, N_SHARD], f32, tag="o")
                    pss = [pp.tile([P, nw], f32, tag=f"ps{ci}", name=f"ps{ci}")
                           for ci, (n0, nw, _) in enumerate(MM_CHUNKS)]
                    mm_sweep(pss, x_t, xr_t, sub)
                    flush(pss, o_t, msi * 2 + sub)

    nc.compile()
    return nc


def _pretile(a, kt_n):
    # [kt_n*P, M] -> [MSUP, P, kt_n, MSW]; element [msi,p,kt,j] = a[kt*P+p, msi*MSW+j]
    return np.ascontiguousarray(a.reshape(kt_n, P, MSUP, MSW).transpose(2, 1, 0, 3))


def _prep_in_maps(x, weight_packed, weight_scale, weight_zero, ckh, ckt):
    x = np.asarray(x, dtype=np.float32)
    wp = np.asarray(weight_packed, dtype=np.int32)
    ws = np.asarray(weight_scale, dtype=np.float32)
    wz = np.asarray(weight_zero, dtype=np.float32)

    xt = np.ascontiguousarray(x.T)           # [K, M] f32
    xh8 = xt.astype(F8)                      # [K, M] fp8 hi part
    xh_tiled = _pretile(xh8, KT)
    kmax = 2 * max(ckh, ckt) * P
    if kmax:
        r8 = (xt[:kmax] - xh8[:kmax].astype(np.float32)).astype(F8)
    if ckh:
        xrh_tiled = np.ascontiguousarray(_pretile(r8[:2 * ckh * P], 2 * ckh)[0:2])
    if ckt:
        xrt_tiled = np.ascontiguousarray(_pretile(r8[:2 * ckt * P], 2 * ckt)[2:MSUP])

    qfull = np.empty((K, N), dtype=F8)
    qfull[0::2] = (wp & 15).astype(F8)
    qfull[1::2] = ((wp >> 4) & 15).astype(F8)
    zs_neg = (-wz * ws).astype(np.float32)

    in_maps = []
    for c in range(N_CORES):
        n0, n1 = c * N_SHARD, (c + 1) * N_SHARD
        # [KP, P, 2, NPAD]: nibbles, then a ones column (rowsum tap), zero pad.
        qc = np.zeros((KP, P, 2, NPAD), dtype=F8)
        qc[:, :, :, :N_SHARD] = (
            qfull[:, n0:n1].reshape(KP, 2, P, N_SHARD).transpose(0, 2, 1, 3))
        qc[:, :, :, ONES_COL] = F8(1.0)
        m = {
            "xh": xh_tiled,
            "q": qc,
            "s": np.ascontiguousarray(ws[n0:n1][None, :]),
            "zs": np.ascontiguousarray(zs_neg[n0:n1][None, :]),
        }
        if ckh:
            m["xrh"] = xrh_tiled
        if ckt:
            m["xrt"] = xrt_tiled
        in_maps.append(m)
    return in_maps


def run(x, weight_packed, weight_scale, weight_zero, trace=False,
        ckh=CKH, ckt=CKT, **spmd_kwargs):
    import time

    from concourse.bass_utils import run_bass_kernel_spmd

    key = (ckh, ckt)
    if key not in _compiled:
        _compiled[key] = _build(ckh, ckt)
    in_maps = _prep_in_maps(x, weight_packed, weight_scale, weight_zero, ckh, ckt)
    last_err = None
    for attempt in range(3):
        try:
            res = run_bass_kernel_spmd(
                _compiled[key], in_maps, core_ids=list(range(N_CORES)), trace=trace,
                **spmd_kwargs,
            )
            break
        except Exception as e:  # transient wedged-device faults recover on retry
            last_err = e
            time.sleep(5)
    else:
        raise last_err
    full = np.concatenate([res.results[c]["out"] for c in range(N_CORES)], axis=1)
    return full, res


def kernel(x, weight_packed, weight_scale, weight_zero):
    full, _ = run(x, weight_packed, weight_scale, weight_zero, trace=False)
    return full


# revision 13
# speedup vs baseline: 1.0426x; 1.0000x over previous
"""Trainium2 Bass kernel for AsymmetricQuantLinear — fp8 DoubleRow + rank-1 zero-point.

    x:             [4096, 4096]  f32
    weight_packed: [2048, 11008] int32 (two 4-bit nibbles per value)
    weight_scale:  [11008] f32
    weight_zero:   [11008] f32
    out = x @ ((unpack(weight_packed) - zero) * scale)   -> [4096, 11008] f32

Tensor-parallel over N across 8 NeuronCores (1376 cols each), x replicated.

Math: out = (x̂ @ q)·s − rowsum(x̂) ⊗ (z·s), with x̂ = x_hi + r on corrected
k-pairs. The nibble values q ∈ [0,15] are exact in fp8 e4m3, so the PE streams
RAW q tiles (no on-device dequant at all); an all-ones column appended to the
moving operand makes the PSUM accumulate rowsum(x̂) for free across the same
start/stop group (hi and residual passes included), and the flush applies the
exact rank-1 zero-point term plus the per-column scale in fp32 on the DVE.

The PE runs fp8 perf_mode=DoubleRow (2 k-planes per instruction, 2
MACs/cell/cycle). x is split x = x_hi + r (both e4m3); residual passes
also accumulate r@q on a subset of k-pairs.

Residual budget is LOPSIDED on purpose: the first two m-superblocks get full
correction (CKH=16 pairs) and the rest get CKT=8, totalling the same 144
corrected pairs (and the same 1.95e-2 global rel err) as a uniform 9 — but
the head's extra residual matmuls are supply-free PE work that lands exactly
inside the startup window where the shared-HBM DMA fill (~10MB of q/x tiles
at ~245GB/s) would otherwise stall the PE.

Startup/tail details: dummy warm-up matmuls on a memset tile keep the PE HAM
clock warm through the initial fill; big DMAs split along the PARTITION axis
(column splits shrink packets and crater DMA throughput); s/zs flush
constants are DMA'd as single rows and partition-broadcast on GpSimd; the
ones-column chunk is issued first within each pass so the rank-1 flush term
is ready earliest; flush + output DMA run per chunk to shrink the tail.

Host prep is layout/precision only: transpose, nibble unpack, fp8/f32 casts,
pre-tiling so every device DMA is one contiguous run per partition.
"""

import numpy as np
import ml_dtypes

M, K, N = 4096, 4096, 11008
N_CORES = 8
N_SHARD = N // N_CORES          # 1376
P = 128
KT = K // P                     # 32 k-tiles
KP = KT // 2                    # 16 k-pairs (DoubleRow consumes 2 k-tiles)
MSW = 256                       # m columns fetched per x DMA (two 128-wide m-tiles)
MSUP = M // MSW                 # 16
NPAD = N_SHARD + 32             # 1408: pad keeps DoubleRow plane stride 32B-aligned
ONES_COL = N_SHARD              # col 1376 of each q tile holds 1.0 -> rowsum(x̂)
# (n0, mm width, flush width); ones-column chunk first so the rowsum tap
# (its PSUM stop) lands before the other chunks' flushes need it.
MM_CHUNKS = [(1024, 354, 352), (0, 512, 512), (512, 512, 512)]
CPAD = (384, 512, 512)
CKH = 16                        # residual k-pairs on m-superblocks 0-1 (head)
CKT = 8                         # residual k-pairs on m-superblocks 2-15 (tail)
WARM_MM = 20                    # dummy warm-up matmuls (constant data, scratch PSUM)

F8 = ml_dtypes.float8_e4m3

_compiled = {}


def _build(ckh, ckt):
    import concourse.mybir as mybir
    import concourse.tile as tile
    from concourse import bacc

    f32 = mybir.dt.float32
    f8 = mybir.dt.float8e4
    DR = mybir.MatmulPerfMode.DoubleRow
    ALU = mybir.AluOpType

    nc = bacc.Bacc("TRN2", target_bir_lowering=False, debug=False, num_devices=N_CORES)
    xh = nc.dram_tensor("xh", [MSUP, P, KT, MSW], f8, kind="ExternalInput").ap()
    if ckh:
        xrh = nc.dram_tensor("xrh", [2, P, 2 * ckh, MSW], f8, kind="ExternalInput").ap()
    if ckt:
        xrt = nc.dram_tensor(
            "xrt", [MSUP - 2, P, 2 * ckt, MSW], f8, kind="ExternalInput").ap()
    q = nc.dram_tensor("q", [KP, P, 2, NPAD], f8, kind="ExternalInput").ap()
    s = nc.dram_tensor("s", [1, N_SHARD], f32, kind="ExternalInput").ap()
    zs = nc.dram_tensor("zs", [1, N_SHARD], f32, kind="ExternalInput").ap()  # -(zero*scale)
    out = nc.dram_tensor("out", [M, N_SHARD], f32, kind="ExternalOutput").ap()

    with tile.TileContext(nc) as tc:
        with (
            tc.tile_pool(name="const", bufs=1) as constp,
            tc.tile_pool(name="wq", bufs=1) as wqp,
            tc.tile_pool(name="xin", bufs=3) as xp,
            tc.tile_pool(name="xrin", bufs=3) as xrp,
            tc.tile_pool(name="ostage", bufs=3) as outp,
            tc.tile_pool(name="psum", space="PSUM", bufs=2) as pp,
        ):
            # --- PE warm-up: matmuls on a memset SBUF tile into a scratch ---
            # PSUM bank. Only dependency is a tiny GpSimd memset, so they
            # issue right after engine start and keep the HAM activity
            # window busy (full 2.4 GHz clock) while the first real tiles
            # stream in. Results are never read.
            warm_w = constp.tile([P, 2, 128], f8, tag="warmw")
            nc.gpsimd.memset(warm_w[:], 1.0)
            warm_ps = pp.tile([P, 128], f32, tag="warm")
            for _ in range(WARM_MM):
                nc.tensor.matmul(
                    warm_ps[:], warm_w[:], warm_w[:],
                    start=True, stop=True, perf_mode=DR,
                )

            # W tiles are the raw q nibbles (exact in fp8) — DMA only, no
            # dequant. A single dma_start is already sprayed across all 16
            # SDMA engines, and efficiency grows with transfer size, so
            # tiles are fetched in the LARGEST units whose arrival still
            # leads consumption: per-chunk for kp0 (gates the first
            # matmul), single tile for kp1, then 3-kp super-tile DMAs.
            # Each dma_start costs ~600ns on its issuing queue, so the
            # startup set is split across BOTH HWDGE queues: q tiles on
            # the SP queue, x / residual / constants on the (otherwise
            # idle) ACT queue, in consumption order.
            w_tiles = [None] * KP

            w0_chunks = []
            for ci, (n0, nw, _) in enumerate(MM_CHUNKS):
                wc = wqp.tile([P, 2, CPAD[ci]], f8, tag=f"w0c{ci}", name="w0c")
                eng = nc.sync if ci == 0 else nc.scalar
                eng.dma_start(wc[:, :, 0:nw], q[0, :, :, n0:n0 + nw])
                w0_chunks.append(wc)

            def w_slice(kp, ci, n0, nw):
                if kp == 0:
                    return w0_chunks[ci][:, :, 0:nw]
                return w_tiles[kp][:, :, n0:n0 + nw]

            def fetch_q(kp0, nkp):
                if nkp == 1:
                    wt = wqp.tile([P, 2, NPAD], f8, tag=f"w{kp0}", name="wt")
                    nc.sync.dma_start(wt[:], q[kp0])
                    w_tiles[kp0] = wt
                    return
                wt = wqp.tile([P, nkp, 2, NPAD], f8, tag=f"w{kp0}", name="wt")
                nc.sync.dma_start(
                    wt[:], q[kp0:kp0 + nkp].rearrange("a p c d -> p a c d"))
                for j in range(nkp):
                    w_tiles[kp0 + j] = wt[:, j]

            x0_t = xp.tile([P, KT, MSW], f8, tag="x", name="x_t")
            x1_t = xp.tile([P, KT, MSW], f8, tag="x", name="x_t")
            if ckh:
                xr0_t = xrp.tile([P, 2 * ckh, MSW], f8, tag="xrh", name="xr_t")
                xr1_t = xrp.tile([P, 2 * ckh, MSW], f8, tag="xrh", name="xr_t")

            def x_batch(t0, te):
                nc.scalar.dma_start(x0_t[:, t0:te, :], xh[0, :, t0:te, :])
                nc.scalar.dma_start(x1_t[:, t0:te, :], xh[1, :, t0:te, :])

            def xr_batch(t0, te):
                if ckh and t0 < 2 * ckh:
                    te = min(te, 2 * ckh)
                    nc.scalar.dma_start(xr0_t[:, t0:te, :], xrh[0, :, t0:te, :])
                    nc.scalar.dma_start(xr1_t[:, t0:te, :], xrh[1, :, t0:te, :])

            x_batch(0, 4)
            fetch_q(1, 1)
            xr_batch(0, 8)
            fetch_q(2, 3)
            x_batch(4, 16)
            fetch_q(5, 3)
            xr_batch(8, 20)
            fetch_q(8, 3)
            x_batch(16, 28)
            fetch_q(11, 3)
            xr_batch(20, KT)
            fetch_q(14, 2)
            x_batch(28, KT)
            # s/zs flush constants: one 5.5KB row each, broadcast on GpSimd.
            s_row = constp.tile([1, N_SHARD], f32, tag="srow")
            zs_row = constp.tile([1, N_SHARD], f32, tag="zsrow")
            nc.scalar.dma_start(s_row[:], s[:])
            nc.scalar.dma_start(zs_row[:], zs[:])
            s_t = constp.tile([P, N_SHARD], f32, tag="s")
            zs_t = constp.tile([P, N_SHARD], f32, tag="zs")
            nc.gpsimd.partition_broadcast(s_t[:], s_row[:])
            nc.gpsimd.partition_broadcast(zs_t[:], zs_row[:])

            def mm_sweep(ps_chunks, x_t, xr_t, sub):
                lhs = lambda t, kp: t[:, 2 * kp:2 * kp + 2, sub * P:(sub + 1) * P]
                for kp in range(KP):
                    for ci, (n0, nw, _) in enumerate(MM_CHUNKS):
                        nc.tensor.matmul(
                            ps_chunks[ci][:],
                            lhs(x_t, kp),
                            w_slice(kp, ci, n0, nw),
                            start=(kp == 0),
                            stop=(kp == KP - 1 and not ckt),
                            perf_mode=DR,
                        )
                for cp in range(ckt):
                    for ci, (n0, nw, _) in enumerate(MM_CHUNKS):
                        nc.tensor.matmul(
                            ps_chunks[ci][:],
                            lhs(xr_t, cp),
                            w_slice(cp, ci, n0, nw),
                            start=False,
                            stop=(cp == ckt - 1),
                            perf_mode=DR,
                        )

            def mm_sweep_interleaved(psss, x_t, xr_t):
                # Both m-subtiles interleaved in one k-sweep, and each
                # corrected pair's residual MMs issued right after its hi MMs:
                # every q-tile arrival unlocks up to 12 queued MMs.
                for kp in range(KP):
                    for sub in (0, 1):
                        lhsT = x_t[:, 2 * kp:2 * kp + 2, sub * P:(sub + 1) * P]
                        for ci, (n0, nw, _) in enumerate(MM_CHUNKS):
                            nc.tensor.matmul(
                                psss[sub][ci][:],
                                lhsT,
                                w_slice(kp, ci, n0, nw),
                                start=(kp == 0),
                                stop=(kp == KP - 1 and not ckh),
                                perf_mode=DR,
                            )
                    if kp < ckh:
                        for sub in (0, 1):
                            lhsT = xr_t[:, 2 * kp:2 * kp + 2, sub * P:(sub + 1) * P]
                            for ci, (n0, nw, _) in enumerate(MM_CHUNKS):
                                nc.tensor.matmul(
                                    psss[sub][ci][:],
                                    lhsT,
                                    w_slice(kp, ci, n0, nw),
                                    start=False,
                                    stop=(kp == ckh - 1),
                                    perf_mode=DR,
                                )

            def flush(ps_chunks, o_t, msub):
                # PSUM -> SBUF per chunk: per-column scale, then the exact
                # rank-1 zero-point term  o += rowsum(x̂) * (-(zero*scale)),
                # with rowsum(x̂) read from the ones-column of chunk 0 (which
                # stops first), then DMA that chunk out immediately.
                rs = ps_chunks[0][:, 352:353]
                m0 = msub * P
                for ci, (n0, _, fw) in enumerate(MM_CHUNKS):
                    nc.vector.tensor_mul(
                        o_t[:, n0:n0 + fw], ps_chunks[ci][:, 0:fw], s_t[:, n0:n0 + fw])
                    nc.vector.scalar_tensor_tensor(
                        o_t[:, n0:n0 + fw], zs_t[:, n0:n0 + fw], rs,
                        o_t[:, n0:n0 + fw], op0=ALU.mult, op1=ALU.add)
                    nc.sync.dma_start(out[m0:m0 + P, n0:n0 + fw], o_t[:, n0:n0 + fw])

            for msi in range(MSUP):
                if msi == 0:
                    x_t, xr_t = x0_t, (xr0_t if ckh else None)
                elif msi == 1:
                    x_t, xr_t = x1_t, (xr1_t if ckh else None)
                else:
                    x_t = xp.tile([P, KT, MSW], f8, tag="x", name="x_t")
                    nc.scalar.dma_start(x_t[:], xh[msi])
                    if ckt:
                        xr_t = xrp.tile([P, 2 * ckt, MSW], f8, tag="xrt", name="xr_t")
                        nc.scalar.dma_start(xr_t[:], xrt[msi - 2])
                    else:
                        xr_t = None
                if msi <= 1:
                    o_ts = [outp.tile([P, N_SHARD], f32, tag="o", name="o_t")
                            for _ in (0, 1)]
                    psss = [
                        [pp.tile([P, nw], f32, tag=f"ps{ci}", name=f"ps{ci}")
                         for ci, (n0, nw, _) in enumerate(MM_CHUNKS)]
                        for _ in (0, 1)
                    ]
                    mm_sweep_interleaved(psss, x_t, xr_t)
                    for sub in (0, 1):
                        flush(psss[sub], o_ts[sub], msi * 2 + sub)
                    continue
                for sub in (0, 1):
                    o_t = outp.tile([P, N_SHARD], f32, tag="o")
                    pss = [pp.tile([P, nw], f32, tag=f"ps{ci}", name=f"ps{ci}")
                           for ci, (n0, nw, _) in enumerate(MM_CHUNKS)]
                    mm_sweep(pss, x_t, xr_t, sub)
                    flush(pss, o_t, msi * 2 + sub)

    nc.compile()
    return nc


def _pretile(a, kt_n):
    # [kt_n*P, M] -> [MSUP, P, kt_n, MSW]; element [msi,p,kt,j] = a[kt*P+p, msi*MSW+j]
    return np.ascontiguousarray(a.reshape(kt_n, P, MSUP, MSW).transpose(2, 1, 0, 3))


def _prep_in_maps(x, weight_packed, weight_scale, weight_zero, ckh, ckt):
    x = np.asarray(x, dtype=np.float32)
    wp = np.asarray(weight_packed, dtype=np.int32)
    ws = np.asarray(weight_scale, dtype=np.float32)
    wz = np.asarray(weight_zero, dtype=np.float32)

    xt = np.ascontiguousarray(x.T)           # [K, M] f32
    xh8 = xt.astype(F8)                      # [K, M] fp8 hi part
    xh_tiled = _pretile(xh8, KT)
    kmax = 2 * max(ckh, ckt) * P
    if kmax:
        r8 = (xt[:kmax] - xh8[:kmax].astype(np.float32)).astype(F8)
    if ckh:
        xrh_tiled = np.ascontiguousarray(_pretile(r8[:2 * ckh * P], 2 * ckh)[0:2])
    if ckt:
        xrt_tiled = np.ascontiguousarray(_pretile(r8[:2 * ckt * P], 2 * ckt)[2:MSUP])

    qfull = np.empty((K, N), dtype=F8)
    qfull[0::2] = (wp & 15).astype(F8)
    qfull[1::2] = ((wp >> 4) & 15).astype(F8)
    zs_neg = (-wz * ws).astype(np.float32)

    in_maps = []
    for c in range(N_CORES):
        n0, n1 = c * N_SHARD, (c + 1) * N_SHARD
        # [KP, P, 2, NPAD]: nibbles, then a ones column (rowsum tap), zero pad.
        qc = np.zeros((KP, P, 2, NPAD), dtype=F8)
        qc[:, :, :, :N_SHARD] = (
            qfull[:, n0:n1].reshape(KP, 2, P, N_SHARD).transpose(0, 2, 1, 3))
        qc[:, :, :, ONES_COL] = F8(1.0)
        m = {
            "xh": xh_tiled,
            "q": qc,
            "s": np.ascontiguousarray(ws[n0:n1][None, :]),
            "zs": np.ascontiguousarray(zs_neg[n0:n1][None, :]),
        }
        if ckh:
            m["xrh"] = xrh_tiled
        if ckt:
            m["xrt"] = xrt_tiled
        in_maps.append(m)
    return in_maps


def run(x, weight_packed, weight_scale, weight_zero, trace=False,
        ckh=CKH, ckt=CKT, **spmd_kwargs):
    import time

    from concourse.bass_utils import run_bass_kernel_spmd

    key = (ckh, ckt)
    if key not in _compiled:
        _compiled[key] = _build(ckh, ckt)
    in_maps = _prep_in_maps(x, weight_packed, weight_scale, weight_zero, ckh, ckt)
    last_err = None
    for attempt in range(3):
        try:
            res = run_bass_kernel_spmd(
                _compiled[key], in_maps, core_ids=list(range(N_CORES)), trace=trace,
                **spmd_kwargs,
            )
            break
        except Exception as e:  # transient wedged-device faults recover on retry
            last_err = e
            time.sleep(5)
    else:
        raise last_err
    full = np.concatenate([res.results[c]["out"] for c in range(N_CORES)], axis=1)
    return full, res


def kernel(x, weight_packed, weight_scale, weight_zero):
    full, _ = run(x, weight_packed, weight_scale, weight_zero, trace=False)
    return full
